# revision 1
# baseline (speedup 1.0000x reference)
"""Trainium2 Bass kernel for CDAttnBlock.

Reference computation (per batch element b, all in fp32):
    q,k,v   = split(x  @ Wqkv)   heads=12, d=64
    q2,k2,v2= split(x2 @ Wqkv)
    o1 = attn(q, k,  v);  o2 = attn(q2, k2, v2);  o3 = attn(q, k2, v2)
    y_i = merge(o_i) @ Wout + bout

Sharding: pure data-parallel over batch (B=8) across 8 NeuronCores;
each core runs the identical program on its own batch element, no
collectives.

Per-core design:
  - All matmul operands are fp16 (full 1 cycle/row PE rate; fp32 is 4x
    slower, fp32r 3x; fp16 keeps end-to-end rel err ~7e-4). PSUM
    accumulation is fp32.
  - x.T built via PE transposes into one [128, 6*1024] fp16 tile
    (hidden on partitions); q.T/k.T per head-pair [128, 1024]; v per
    s-tile [128, 12*65] with a ones column per head so the a@v matmul
    also emits the softmax denominator row.
  - Scores are computed transposed (sT = kT.T @ qT) so softmax needs no
    on-chip transposes; exp runs on ScalarE straight out of PSUM with
    the 1/sqrt(d) scale folded in; no max-subtraction (scores ~N(0,1)).
  - Denominators from 4 heads are collected at partitions {0,32,64,96},
    inverted in ONE DVE reciprocal (its cost scales only with the free
    size), broadcast to 64 partitions on the idle GpSimd engine
    (partition_broadcast), and applied with a fp16 DVE multiply.
  - o accumulates transposed so the output projection (oT as stationary
    operand) yields y in natural [1024, 768] layout.
  - The attention phases are ScalarE(exp)-bound; leaving the PE idle
    there makes the HAM clock-gate drop it to 1.2 GHz. So all other PE
    work (x2 transposes + its qkv projections, then the output
    projections) is chopped into small thunks and interleaved into the
    attention instruction stream to keep the PE continuously busy.
"""

import numpy as np

import concourse.bass as bass
import concourse.tile as tile
from concourse import bacc, mybir
from concourse.bass_utils import run_bass_kernel_spmd
from concourse.masks import make_identity

F32 = mybir.dt.float32
F16 = mybir.dt.float16
AF = mybir.ActivationFunctionType

HIDDEN = 768
HEADS = 12
D = 64
S = 1024
B = 8
SCALE = D ** -0.5
NPAIR = HEADS // 2          # 6 head pairs
KT = HIDDEN // 128          # 6 k-tiles over hidden
ST = S // 128               # 8 s-tiles
VW = D + 1                  # 65: v columns + ones column


class Ctx:
    """Shared handles for the kernel builder."""


def _emit_xt(c, x_ap, xT, xnat, psum_pool, psum_tag, thunks=None):
    """Build xT [128, KT*S] fp16 from x [S, H]: DMA natural tiles, PE
    transpose 6 blocks per s-tile into one psum tile, one DVE evac."""
    nc = c.nc
    out3 = xT.rearrange("p (h s) -> p h s", s=S)
    xns = {}

    def dma(st):
        def f():
            xn = xnat.tile([128, HIDDEN], F32, name="xn", tag="xn")
            xns[st] = xn
            nc.sync.dma_start(xn[:], x_ap[st * 128:(st + 1) * 128, :])
        return f

    def tp(st, half):
        def f():
            tag = psum_tag[half % len(psum_tag)] if isinstance(
                psum_tag, (list, tuple)) else psum_tag
            pt = psum_pool.tile([128, 3 * 128], F32, name="tpp", tag=tag)
            for i in range(3):
                ht = 3 * half + i
                nc.tensor.transpose(
                    pt[:, i * 128:(i + 1) * 128],
                    xns[st][:, ht * 128:(ht + 1) * 128], c.ident[:])
            nc.vector.tensor_copy(
                out3[:, 3 * half:3 * half + 3, st * 128:(st + 1) * 128],
                pt.rearrange("p (h s) -> p h s", s=128))
        return f

    for st in range(ST):
        for f in (dma(st), tp(st, 0), tp(st, 1)):
            if thunks is None:
                f()
            else:
                thunks.append(f)


def _emit_qkv(c, xT, qT, kT, v_st, psum_pool, psum_tag, thunks=None,
              parts=("v", "q", "k")):
    """xT [128, KT*S] fp16 -> qT/kT per pair [128, S] fp16 and v per
    s-tile [128, 12*65] fp16 (with ones column). `parts` selects which
    of v/q/k to emit."""
    nc = c.nc

    def xts(kt, a, b):
        return xT[:, kt * S + a:kt * S + b]

    # ---- v: out [s-tile, 768] accumulated over kt ----
    def v_half(st, half):
        def f():
            tag = psum_tag[half % len(psum_tag)] if isinstance(
                psum_tag, (list, tuple)) else psum_tag
            lo, hi = (0, 512) if half == 0 else (512, 768)
            vp = psum_pool.tile([128, hi - lo], F32, name="vp", tag=tag)
            for kt in range(KT):
                nc.tensor.matmul(
                    vp[:], xts(kt, st * 128, (st + 1) * 128),
                    c.wq16[kt][:, 2 * HIDDEN + lo:2 * HIDDEN + hi],
                    start=(kt == 0), stop=(kt == KT - 1))
            vs3 = v_st[st].rearrange("p (h w) -> p h w", w=VW)
            ha, hb = (0, 8) if half == 0 else (8, 12)
            nc.vector.tensor_copy(
                vs3[:, ha:hb, 0:D],
                vp.rearrange("p (h w) -> p h w", w=D))
            if half == 1:
                nc.vector.tensor_copy(
                    vs3[:, :, D:VW],
                    c.onescol[:, None, :].broadcast_to([128, HEADS, 1]))
        return f

    if "v" in parts:
        for st in range(ST):
            for half in range(2):
                f = v_half(st, half)
                if thunks is None:
                    f()
                else:
                    thunks.append(f)

    # ---- qT / kT per pair: lhsT = Wq/Wk col slice, rhs = xT ----
    def qk_half(p, base, dst, half, hold):
        def f():
            tag = psum_tag[half % len(psum_tag)] if isinstance(
                psum_tag, (list, tuple)) else psum_tag
            pp = psum_pool.tile([128, 512], F32, name="qkp", tag=tag)
            lo = half * 512
            for kt in range(KT):
                nc.tensor.matmul(
                    pp[:],
                    c.wq16[kt][:, base + p * 128:base + (p + 1) * 128],
                    xts(kt, lo, lo + 512),
                    start=(kt == 0), stop=(kt == KT - 1))
            nc.vector.tensor_copy(dst[p][:, lo:lo + 512], pp[:])
        return f

    sel = [(0, qT)] * ("q" in parts) + [(HIDDEN, kT)] * ("k" in parts)
    for p in range(NPAIR):
        for base, dst in sel:
            hold = {}
            for half in range(2):
                f = qk_half(p, base, dst, half, hold)
                if thunks is None:
                    f()
                else:
                    thunks.append(f)


def _emit_proj(c, oT, y_dram, psum_pool, psum_tag, thunks=None):
    """y = oT.T @ Wout + bias -> DRAM, natural [S, H] layout."""
    nc = c.nc

    def half(st, h, hold):
        def f():
            tag = psum_tag[h % len(psum_tag)] if isinstance(
                psum_tag, (list, tuple)) else psum_tag
            lo, hi = (0, 512) if h == 0 else (512, 768)
            yp = psum_pool.tile([128, hi - lo], F32, name="yp", tag=tag)
            for ct in range(KT):
                nc.tensor.matmul(
                    yp[:], oT[ct][:, st * 128:(st + 1) * 128],
                    c.wout16[ct][:, lo:hi],
                    start=(ct == 0), stop=(ct == KT - 1))
            if h == 0:
                hold["yt"] = c.ysb.tile([128, HIDDEN], F32, name="yt",
                                        tag="yt")
            yt = hold["yt"]
            nc.vector.tensor_add(yt[:, lo:hi], yp[:],
                                 c.bias_sb[:, lo:hi])
            if h == 1:
                nc.sync.dma_start(y_dram[st * 128:(st + 1) * 128, :], yt[:])
        return f

    for st in range(ST):
        hold = {}
        for h in range(2):
            f = half(st, h, hold)
            if thunks is None:
                f()
            else:
                thunks.append(f)


def _attn(c, tc, qT, kT, v_st, oT, thunks, npump=None):
    """One attention (12 heads, one at a time). `thunks` (aux PE work +
    deferred normalize chains) are pumped into the exp-wait gaps at an
    even pace so the PE stays busy (and the HAM clock-gate warm) for the
    whole phase."""
    nc = c.nc
    work = list(thunks)          # PE-heavy aux thunks
    dveq = []                    # deferred normalize (DVE/gpsimd only)
    state = {"i": 0, "credit": 0.0, "j": 0}
    # pump sites: one per kt per head (96) plus one per head tail (12)
    sites = HEADS * ST + HEADS
    rate = None

    def pump(k=1.0):
        nonlocal rate
        if rate is None:
            rate = max(1.0, len(work) / sites)
        state["credit"] += k * rate
        while state["credit"] >= 1.0 and state["i"] < len(work):
            work[state["i"]]()
            state["i"] += 1
            state["credit"] -= 1.0

    def pump_dve(n=1):
        for _ in range(n):
            if state["j"] < len(dveq):
                dveq[state["j"]]()
                state["j"] += 1

    def drain():
        while state["i"] < len(work):
            work[state["i"]]()
            state["i"] += 1
        while state["j"] < len(dveq):
            dveq[state["j"]]()
            state["j"] += 1

    sps = tc.alloc_tile_pool(name="sps", bufs=2, space="PSUM")
    ovps = tc.alloc_tile_pool(name="ovps", bufs=1, space="PSUM")
    exps = tc.alloc_tile_pool(name="exps", bufs=4, side="right")
    ovstage = tc.alloc_tile_pool(name="ovstage", bufs=5, side="right")
    smalls = tc.alloc_tile_pool(name="smalls", bufs=2, side="right")

    def normalize_thunks(den4, ovs4, g):
        """Deferred DVE/gpsimd normalize chain for one 4-head group."""
        hold = {}

        def t_recip():
            recf = smalls.tile([128, S], F32, name="recf", tag="recf")
            nc.vector.reciprocal(recf[:], den4[:])
            hold["recf"] = recf

        def t_head(gi):
            def f():
                h = 4 * g + gi
                p, hh = h // 2, h % 2
                hp = slice(hh * D, (hh + 1) * D)
                rrow = smalls.tile([1, S], F16, name="rrow", tag="rrow",
                                   bufs=2)
                nc.vector.tensor_copy(
                    rrow[:], hold["recf"][32 * gi:32 * gi + 1, :])
                bcs = smalls.tile([D, S], F16, name="bcs", tag="bcs",
                                  bufs=2)
                nc.gpsimd.partition_broadcast(bcs[:], rrow[:])
                nc.vector.tensor_mul(oT[p][hp, :], ovs4[gi][:], bcs[:])
            return f

        return [t_recip] + [t_head(gi) for gi in range(4)]

    for g in range(HEADS // 4):       # 3 groups of 4 heads
        den4 = smalls.tile([128, S], F32, name="den4", tag="den4")
        nc.vector.memset(den4[:], 1.0)
        ovs4 = []
        for gi in range(4):
            h = 4 * g + gi
            p, hh = h // 2, h % 2
            hp = slice(hh * D, (hh + 1) * D)
            ov = ovps.tile([VW, S], F32, name="ov", tag="ov")
            for kt in range(ST):
                sp = sps.tile([128, S], F32, name="sp", tag="sp")
                kts = kT[p][hp, kt * 128:(kt + 1) * 128]
                for nb in range(2):
                    nc.tensor.matmul(
                        sp[:, nb * 512:(nb + 1) * 512], kts,
                        qT[p][hp, nb * 512:(nb + 1) * 512],
                        start=True, stop=True)
                ex = exps.tile([128, S], F16, name="ex", tag="ex")
                nc.scalar.activation(ex[:], sp[:], AF.Exp,
                                     bias=c.zbias[:], scale=SCALE)
                vs = v_st[kt].rearrange("q (h w) -> q h w", w=VW)[:, h, :]
                for nb in range(2):
                    nc.tensor.matmul(
                        ov[:, nb * 512:(nb + 1) * 512], vs,
                        ex[:, nb * 512:(nb + 1) * 512],
                        start=(kt == 0), stop=(kt == ST - 1))
                # front-load aux work within the head: the last kts stay
                # clear so the staging copies that release the ov PSUM
                # slot aren't queued behind aux DVE evacs
                pump(4.0 / 3.0 if kt < 6 else 0.0)
                if kt < 3:
                    pump_dve(1)
            nc.vector.tensor_copy(den4[32 * gi:32 * gi + 1, :],
                                  ov[D:VW, :])
            ovs = ovstage.tile([D, S], F16, name="ovs", tag="ovs")
            nc.vector.tensor_copy(ovs[:], ov[0:D, :])
            ovs4.append(ovs)
            pump(1.0)
        dveq.extend(normalize_thunks(den4, ovs4, g))
    drain()
    smalls.release()
    ovstage.release()
    exps.release()
    ovps.release()
    sps.release()


def build_kernel(ctx, tc, x, x2, wq, wo, bo, y1, y2, y3):
    nc = tc.nc
    c = Ctx()
    c.nc = nc

    # ---------------- constants + weights (fp16 resident) -------------
    const = ctx.enter_context(tc.tile_pool(name="const", bufs=1))
    c.ident = const.tile([128, 128], F32, name="ident")
    make_identity(nc, c.ident)
    c.zbias = const.tile([128, 1], F32, name="zbias")
    nc.vector.memset(c.zbias[:], 0.0)
    c.onescol = const.tile([128, 1], F32, name="onescol")
    nc.vector.memset(c.onescol[:], 1.0)
    c.bias_sb = const.tile([128, HIDDEN], F32, name="bias_sb")
    bo_bcast = bass.AP(tensor=bo.tensor, offset=bo.offset,
                       ap=[[0, 128]] + list(bo.ap))
    nc.sync.dma_start(c.bias_sb[:], bo_bcast)

    wstage = tc.alloc_tile_pool(name="wstage", bufs=2, side="right")
    woutp = ctx.enter_context(tc.tile_pool(name="woutp", bufs=1))
    c.wout16 = []
    for ct in range(KT):
        f = wstage.tile([128, HIDDEN], F32, name="wof", tag="wof")
        nc.sync.dma_start(f[:], wo[ct * 128:(ct + 1) * 128, :])
        t = woutp.tile([128, HIDDEN], F16, name=f"wout{ct}", tag=f"wout{ct}")
        nc.vector.tensor_copy(t[:], f[:])
        c.wout16.append(t)

    def persist(pool, shape, base, n, dtype=F16):
        return [pool.tile(shape, dtype, name=f"{base}{i}", tag=f"{base}{i}")
                for i in range(n)]

    qxp = ctx.enter_context(tc.tile_pool(name="qxp", bufs=1))
    qT_x = persist(qxp, [128, S], "qTx", NPAIR)
    # x2's qkv allocated before kvxp so kvxp can release first (LIFO)
    kvx2p = ctx.enter_context(tc.tile_pool(name="kvx2p", bufs=1))
    qT_x2 = persist(kvx2p, [128, S], "qTx2", NPAIR)
    kT_x2 = persist(kvx2p, [128, S], "kTx2", NPAIR)
    v_x2 = persist(kvx2p, [128, HEADS * VW], "vx2", ST)
    # oT slots: tag "oTa" holds oT1 then oT3; "oTb" holds oT2
    otp = ctx.enter_context(tc.tile_pool(name="otp", bufs=1))
    oT1 = persist(otp, [128, S], "oTa", NPAIR)
    kvxp = tc.alloc_tile_pool(name="kvxp", bufs=1)
    kT_x = persist(kvxp, [128, S], "kTx", NPAIR)
    v_x = persist(kvxp, [128, HEADS * VW], "vx", ST)
    wqp = tc.alloc_tile_pool(name="wqp", bufs=1)
    c.wq16 = []
    for kt in range(KT):
        f = wstage.tile([128, 3 * HIDDEN], F32, name="wqf", tag="wqf")
        nc.sync.dma_start(f[:], wq[kt * 128:(kt + 1) * 128, :])
        t = wqp.tile([128, 3 * HIDDEN], F16, name=f"wq16{kt}",
                     tag=f"wq16{kt}")
        nc.vector.tensor_copy(t[:], f[:])
        c.wq16.append(t)
    wstage.release()
    c.ysb = ctx.enter_context(tc.tile_pool(name="ysb", bufs=2, side="right"))

    # ---------------- phase 1: xT + qkv(x), dense ---------------------
    xnat = tc.alloc_tile_pool(name="xnat", bufs=2, side="right")
    p1ps = tc.alloc_tile_pool(name="p1ps", bufs=2, space="PSUM")
    xtp = tc.alloc_tile_pool(name="xtp", bufs=1)
    xT = xtp.tile([128, KT * S], F16, name="xT")
    _emit_xt(c, x, xT, xnat, p1ps, "p1")
    _emit_qkv(c, xT, qT_x, kT_x, v_x, p1ps, "p1")
    xtp.release()
    p1ps.release()

    # ---- phase 2: attn(o1), aux = x2T + k2 + v2 ----------------------
    auxp = tc.alloc_tile_pool(name="auxp", bufs=1, space="PSUM")
    AUXT = ["auxA", "auxB"]
    x2tp = tc.alloc_tile_pool(name="x2tp", bufs=1)
    x2T = x2tp.tile([128, KT * S], F16, name="x2T")
    thunks2 = []
    _emit_xt(c, x2, x2T, xnat, auxp, AUXT, thunks=thunks2)
    _emit_qkv(c, x2T, qT_x2, kT_x2, v_x2, auxp, AUXT, thunks=thunks2,
              parts=("v", "k"))
    _attn(c, tc, qT_x, kT_x, v_x, oT1, thunks2)

    # ---- phase 3: attn(o3), aux = q2 + proj(y1) ----------------------
    oT3 = persist(otp, [128, S], "oTb", NPAIR)
    thunks3 = []
    _emit_qkv(c, x2T, qT_x2, kT_x2, v_x2, auxp, AUXT, thunks=thunks3,
              parts=("q",))
    _emit_proj(c, oT1, y1, auxp, AUXT, thunks=thunks3)
    _attn(c, tc, qT_x, kT_x2, v_x2, oT3, thunks3)
    x2tp.release()
    wqp.release()
    kvxp.release()

    # ---- phase 4: attn(o2), aux = proj(y3) ---------------------------
    oT2 = persist(otp, [128, S], "oTa", NPAIR)
    thunks4 = []
    _emit_proj(c, oT3, y3, auxp, AUXT, thunks=thunks4)
    _attn(c, tc, qT_x2, kT_x2, v_x2, oT2, thunks4)

    # ---- phase 5: proj(y2) -------------------------------------------
    _emit_proj(c, oT2, y2, auxp, AUXT)
    auxp.release()
    xnat.release()


def build_bass():
    from contextlib import ExitStack
    nc = bacc.Bacc("TRN2", target_bir_lowering=False, debug=False,
                   num_devices=B)
    x = nc.dram_tensor("x", [S, HIDDEN], F32, kind="ExternalInput").ap()
    x2 = nc.dram_tensor("x2", [S, HIDDEN], F32, kind="ExternalInput").ap()
    wq = nc.dram_tensor("Wqkv", [HIDDEN, 3 * HIDDEN], F32,
                        kind="ExternalInput").ap()
    wo = nc.dram_tensor("Wout", [HIDDEN, HIDDEN], F32,
                        kind="ExternalInput").ap()
    bo = nc.dram_tensor("bout", [HIDDEN], F32, kind="ExternalInput").ap()
    y1 = nc.dram_tensor("y1", [S, HIDDEN], F32, kind="ExternalOutput").ap()
    y2 = nc.dram_tensor("y2", [S, HIDDEN], F32, kind="ExternalOutput").ap()
    y3 = nc.dram_tensor("y3", [S, HIDDEN], F32, kind="ExternalOutput").ap()
    with tile.TileContext(nc) as tc:
        with ExitStack() as ctx:
            build_kernel(ctx, tc, x, x2, wq, wo, bo, y1, y2, y3)
    nc.compile()
    return nc


_NC_CACHE = []


def kernel(x, x2, Wqkv, Wout, bout):
    if not _NC_CACHE:
        _NC_CACHE.append(build_bass())
    nc = _NC_CACHE[0]
    in_maps = [
        {"x": np.ascontiguousarray(x[b]), "x2": np.ascontiguousarray(x2[b]),
         "Wqkv": Wqkv, "Wout": Wout, "bout": bout}
        for b in range(B)
    ]
    res = run_bass_kernel_spmd(nc, in_maps, list(range(B)))
    y1 = np.stack([res.results[b]["y1"] for b in range(B)])
    y2 = np.stack([res.results[b]["y2"] for b in range(B)])
    y3 = np.stack([res.results[b]["y3"] for b in range(B)])
    return (y1, y2, y3)



# revision 15
# speedup vs baseline: 1.2651x; 1.2651x over previous
"""Trainium2 Bass kernel for CDAttnBlock (v2 — pipelined).

Reference computation (per batch element b, all in fp32):
    q,k,v   = split(x  @ Wqkv)   heads=12, d=64
    q2,k2,v2= split(x2 @ Wqkv)
    o1 = attn(q, k,  v);  o2 = attn(q2, k2, v2);  o3 = attn(q, k2, v2)
    y_i = merge(o_i) @ Wout + bout

Sharding: pure data-parallel over batch (B=8) across 8 NeuronCores.

v2 design (vs v1 baseline at ~700us):
  - ScalarE exp stream (288 ACTs x ~1.15us = the metronome) starts ~30us
    in (right after x DMA + xT + pair-0 q/k priming) and never drains
    between attention phases: all other work (qkv of both inputs, x2
    transposes, output projections, softmax normalization) is pumped as
    small thunks into the per-step gaps.
  - Scores are computed per (pair, q-half, key-tile) with the two heads
    of a pair ROW-TILED onto disjoint PE quadrants (contraction d=64 at
    partitions 0-63 / 64-127) so both heads' score matmuls run
    concurrently; one [128,1024] PSUM tile holds [headA | headB] halves
    and one exp covers both.
  - av keeps the ones-column trick (lhsT [128, 65], row 64 = softmax
    denominator) accumulating over key-tiles into [65, 512] PSUM.
  - Normalize: denominators for all 12 heads collected into one
    [12, 1024] f16 tile -> ONE DVE reciprocal per attention (recip is
    ~6.4 cyc/elem so batching frees ~40us) -> gpsimd broadcast ->
    in-place f16 multiply on oT.
  - PSUM: scores 2x[128,1024] (4 banks) + ov 2x[65,512] (2) + aux 2x
    [128,512] (2) = 8 banks.
"""

import numpy as np

import concourse.bass as bass
import concourse.tile as tile
from concourse import bacc, mybir
from concourse.bass_utils import run_bass_kernel_spmd
from concourse.masks import make_identity

F32 = mybir.dt.float32
F16 = mybir.dt.float16
AF = mybir.ActivationFunctionType

HIDDEN = 768
HEADS = 12
D = 64
S = 1024
B = 8
SCALE = D ** -0.5
NPAIR = HEADS // 2          # 6 head pairs
KT = HIDDEN // 128          # 6 k-tiles over hidden
ST = S // 128               # 8 s-tiles
VW = D + 1                  # 65: v columns + ones column


class Ctx:
    """Shared handles for the kernel builder."""


# ---------------------------------------------------------------------------
# aux thunk builders (each returns a list of closures; every closure is a
# small burst of engine work suitable for pumping into exp-stream gaps)
# ---------------------------------------------------------------------------

def th_dma_x(c, x_ap, st, xns, key):
    nc = c.nc

    def f():
        xn = c.xnat.tile([128, HIDDEN], F32, name="xn", tag="xn")
        xns[(key, st)] = xn
        nc.sync.dma_start(xn[:], x_ap[st * 128:(st + 1) * 128, :])
    return [f]


def th_xt(c, xT, st, xns, key):
    """PE-transpose one s-tile of x into xT [128, KT*S]; two thunks."""
    nc = c.nc
    out3 = xT.rearrange("p (h s) -> p h s", s=S)

    def tp(half):
        def f():
            pt = c.auxp.tile([128, 3 * 128], F32, name="tpp",
                             tag=c.aux_tag())
            for i in range(3):
                ht = 3 * half + i
                nc.tensor.transpose(
                    pt[:, i * 128:(i + 1) * 128],
                    xns[(key, st)][:, ht * 128:(ht + 1) * 128], c.ident[:])
            nc.vector.tensor_copy(
                out3[:, 3 * half:3 * half + 3, st * 128:(st + 1) * 128],
                pt.rearrange("p (h s) -> p h s", s=128))
        return f
    return [tp(0), tp(1)]


def th_v_half(c, xT, v_st, st, half):
    """v columns for heads [0..8) (half 0) or [8..12) (half 1) of s-tile."""
    nc = c.nc

    def f():
        lo, hi = (0, 512) if half == 0 else (512, 768)
        vp = c.auxp.tile([128, hi - lo], F32, name="vp", tag=c.aux_tag())
        for kt in range(KT):
            nc.tensor.matmul(
                vp[:], xT[:, kt * S + st * 128:kt * S + (st + 1) * 128],
                c.wq16[kt][:, 2 * HIDDEN + lo:2 * HIDDEN + hi],
                start=(kt == 0), stop=(kt == KT - 1))
        vs3 = v_st[st].rearrange("p (h w) -> p h w", w=VW)
        ha, hb = (0, 8) if half == 0 else (8, 12)
        nc.vector.tensor_copy(
            vs3[:, ha:hb, 0:D], vp.rearrange("p (h w) -> p h w", w=D))
        if half == 1:
            nc.vector.tensor_copy(
                vs3[:, :, D:VW],
                c.onescol[:, None, :].broadcast_to([128, HEADS, 1]))
    return [f]


def th_qk(c, xT, p, base, dst):
    """qT or kT for one pair: two half-thunks, each [128, 512] psum."""
    nc = c.nc

    def half(hf):
        def f():
            pp = c.auxp.tile([128, 512], F32, name="qkp", tag=c.aux_tag())
            lo = hf * 512
            for kt in range(KT):
                nc.tensor.matmul(
                    pp[:],
                    c.wq16[kt][:, base + p * 128:base + (p + 1) * 128],
                    xT[:, kt * S + lo:kt * S + lo + 512],
                    start=(kt == 0), stop=(kt == KT - 1))
            nc.vector.tensor_copy(dst[p][:, lo:lo + 512], pp[:])
        return f
    return [half(0), half(1)]


def th_proj(c, oT, y_dram, st):
    """y[st] = oT.T @ Wout + bias -> DRAM; two half-thunks."""
    nc = c.nc
    hold = {}

    def half(h):
        def f():
            lo, hi = (0, 512) if h == 0 else (512, 768)
            yp = c.auxp.tile([128, hi - lo], F32, name="yp", tag=c.aux_tag())
            for ct in range(KT):
                nc.tensor.matmul(
                    yp[:], oT[ct][:, st * 128:(st + 1) * 128],
                    c.wout16[ct][:, lo:hi],
                    start=(ct == 0), stop=(ct == KT - 1))
            if h == 0:
                hold["yt"] = c.ysb.tile([128, HIDDEN], F32, name="yt",
                                        tag="yt")
            yt = hold["yt"]
            nc.vector.tensor_add(yt[:, lo:hi], yp[:], c.bias_sb[:, lo:hi])
            if h == 1:
                nc.sync.dma_start(y_dram[st * 128:(st + 1) * 128, :], yt[:])
        return f
    return [half(0), half(1)]


def th_norm(c, oT, den4s, attn_id):
    """Deferred normalize chain for one attention (12 heads in 3 groups
    of 4): per group, upcast + fast-approx reciprocal + downcast, then
    per-head gpsimd bcast + in-place f16 multiply on oT."""
    nc = c.nc
    hold = {}

    def t_recip(g):
        def f():
            df = c.dnp.tile([128, S], F32, name="df", tag="df", bufs=1)
            nc.vector.tensor_copy(df[:], den4s[g][:])
            nc.vector.reciprocal_approx_fast(df[:], df[:])
            r16 = c.dnp.tile([128, S], F16, name="r16", tag=f"r16{g}",
                             bufs=1)
            nc.vector.tensor_copy(r16[:], df[:])
            hold[g] = r16
        return f

    def t_head(h):
        def f():
            p, hh = h // 2, h % 2
            hp = slice(hh * D, (hh + 1) * D)
            g, j = h // 4, h % 4
            # bcs half matches oT's base partition (SBUF tensor_tensor
            # requires equal input base partitions)
            bcs = c.bcsp.tile([128, S], F16, name="bcs", tag="bcs")
            nc.gpsimd.partition_broadcast(
                bcs[hp, :], hold[g][32 * j:32 * j + 1, :])
            nc.vector.tensor_mul(oT[p][hp, :], oT[p][hp, :], bcs[hp, :])
        return f

    out = []
    for g in range(3):
        out.append(t_recip(g))
        out += [t_head(4 * g + j) for j in range(4)]
    return out


# ---------------------------------------------------------------------------
# the attention pipeline
# ---------------------------------------------------------------------------

def attention(c, qT, kT, v_st, oT, den4s, work, dveq):
    """One attention (12 heads as 6 row-tiled pairs x 2 q-halves x 8
    key-tiles). `work` = PE-ish aux thunks, `dveq` = deferred normalize
    chains of the previous attention; both are pumped into the gaps."""
    nc = c.nc
    state = {"i": 0, "credit": 0.0, "j": 0}
    sites = NPAIR * 2 * ST
    rate = max(0.001, len(work) / sites)

    def pump(k=1.0):
        state["credit"] += k * rate
        while state["credit"] >= 1.0 and state["i"] < len(work):
            work[state["i"]]()
            state["i"] += 1
            state["credit"] -= 1.0

    def pump_dve(n=1):
        for _ in range(n):
            if state["j"] < len(dveq):
                dveq[state["j"]]()
                state["j"] += 1

    for pair in range(NPAIR):
        for qh in range(2):
            qsl = slice(qh * 512, (qh + 1) * 512)
            ovA = c.ovps.tile([VW, 512], F32, name="ovA", tag="ovA")
            ovB = c.ovps.tile([VW, 512], F32, name="ovB", tag="ovB")
            for kt in range(ST):
                sp = c.sps.tile([128, S], F32, name="sp", tag="sp")
                ksl = slice(kt * 128, (kt + 1) * 128)
                nc.tensor.matmul(sp[:, 0:512], kT[pair][0:D, ksl],
                                 qT[pair][0:D, qsl], start=True, stop=True)
                nc.tensor.matmul(sp[:, 512:1024], kT[pair][D:128, ksl],
                                 qT[pair][D:128, qsl], start=True, stop=True)
                ex = c.exps.tile([128, S], F16, name="ex", tag="ex")
                nc.scalar.activation(ex[:], sp[:], AF.Exp,
                                     bias=c.zbias[:], scale=SCALE)
                vs3 = v_st[kt].rearrange("q (h w) -> q h w", w=VW)
                nc.tensor.matmul(ovA[:], vs3[:, 2 * pair, :], ex[:, 0:512],
                                 start=(kt == 0), stop=(kt == ST - 1))
                nc.tensor.matmul(ovB[:], vs3[:, 2 * pair + 1, :],
                                 ex[:, 512:1024],
                                 start=(kt == 0), stop=(kt == ST - 1))
                if kt < 4:
                    pump_dve(1)
                pump(1.0)
            # sweep tail: evacuate o (f16, pre-normalize) + denominators
            # (den row h goes to partition 32*(h%4) of group tile h//4 —
            # DVE moves must keep partition start congruent mod 32)
            hA, hB = 2 * pair, 2 * pair + 1
            nc.vector.tensor_copy(oT[pair][0:D, qsl], ovA[0:D, :])
            nc.vector.tensor_copy(
                den4s[hA // 4][32 * (hA % 4):32 * (hA % 4) + 1, qsl],
                ovA[D:VW, :])
            nc.vector.tensor_copy(oT[pair][D:128, qsl], ovB[0:D, :])
            nc.vector.tensor_copy(
                den4s[hB // 4][32 * (hB % 4):32 * (hB % 4) + 1, qsl],
                ovB[D:VW, :])
    # drain leftovers
    while state["i"] < len(work):
        work[state["i"]]()
        state["i"] += 1
    while state["j"] < len(dveq):
        dveq[state["j"]]()
        state["j"] += 1


def build_kernel(ctx, tc, x, x2, wq, wo, bo, y1, y2, y3):
    nc = tc.nc
    c = Ctx()
    c.nc = nc
    c._aux_flip = [0]

    def aux_tag():
        c._aux_flip[0] ^= 1
        return ("auxA", "auxB")[c._aux_flip[0]]
    c.aux_tag = aux_tag

    # ---------------- constants ---------------------------------------
    const = ctx.enter_context(tc.tile_pool(name="const", bufs=1))
    c.ident = const.tile([128, 128], F32, name="ident")
    make_identity(nc, c.ident)
    c.zbias = const.tile([128, 1], F32, name="zbias")
    nc.vector.memset(c.zbias[:], 0.0)
    c.onescol = const.tile([128, 1], F32, name="onescol")
    nc.vector.memset(c.onescol[:], 1.0)
    c.bias_sb = const.tile([128, HIDDEN], F32, name="bias_sb")

    # ---------------- persistent pools --------------------------------
    woutp = ctx.enter_context(tc.tile_pool(name="woutp", bufs=1))
    c.wout16 = [woutp.tile([128, HIDDEN], F16, name=f"wout{ct}",
                           tag=f"wout{ct}") for ct in range(KT)]
    qxp = ctx.enter_context(tc.tile_pool(name="qxp", bufs=1))
    qT_x = [qxp.tile([128, S], F16, name=f"qTx{i}", tag=f"qTx{i}")
            for i in range(NPAIR)]
    kvx2p = ctx.enter_context(tc.tile_pool(name="kvx2p", bufs=1))
    qT_x2 = [kvx2p.tile([128, S], F16, name=f"qTx2{i}", tag=f"qTx2{i}")
             for i in range(NPAIR)]
    kT_x2 = [kvx2p.tile([128, S], F16, name=f"kTx2{i}", tag=f"kTx2{i}")
             for i in range(NPAIR)]
    v_x2 = [kvx2p.tile([128, HEADS * VW], F16, name=f"vx2{i}",
                       tag=f"vx2{i}") for i in range(ST)]
    otp = ctx.enter_context(tc.tile_pool(name="otp", bufs=1))
    oT1 = [otp.tile([128, S], F16, name=f"oTa{i}", tag=f"oTa{i}")
           for i in range(NPAIR)]

    # pools released mid-build (allocated after the persistent ones)
    x2tp = tc.alloc_tile_pool(name="x2tp", bufs=1)
    x2T = x2tp.tile([128, KT * S], F16, name="x2T")
    kvxp = tc.alloc_tile_pool(name="kvxp", bufs=1)
    kT_x = [kvxp.tile([128, S], F16, name=f"kTx{i}", tag=f"kTx{i}")
            for i in range(NPAIR)]
    v_x = [kvxp.tile([128, HEADS * VW], F16, name=f"vx{i}", tag=f"vx{i}")
           for i in range(ST)]
    wqp = tc.alloc_tile_pool(name="wqp", bufs=1)
    c.wq16 = [wqp.tile([128, 3 * HIDDEN], F16, name=f"wq16{kt}",
                       tag=f"wq16{kt}") for kt in range(KT)]
    xtp = tc.alloc_tile_pool(name="xtp", bufs=1)
    xT = xtp.tile([128, KT * S], F16, name="xT")

    # ---------------- working pools (right side) ----------------------
    c.xnat = tc.alloc_tile_pool(name="xnat", bufs=2, side="right")
    c.exps = tc.alloc_tile_pool(name="exps", bufs=2, side="right")
    c.dnp = tc.alloc_tile_pool(name="dnp", bufs=2, side="right")
    c.bcsp = tc.alloc_tile_pool(name="bcsp", bufs=1, side="right")
    c.ysb = tc.alloc_tile_pool(name="ysb", bufs=2, side="right")
    wstage = tc.alloc_tile_pool(name="wstage", bufs=2, side="right")

    # ---------------- PSUM pools --------------------------------------
    c.sps = tc.alloc_tile_pool(name="sps", bufs=2, space="PSUM")
    c.ovps = tc.alloc_tile_pool(name="ovps", bufs=1, space="PSUM")
    c.auxp = tc.alloc_tile_pool(name="auxp", bufs=1, space="PSUM")

    # ---------------- lead-in: DMA order + priming --------------------
    # x first (gates everything), then Wqkv, then Wout/bias, then x2.
    xns = {}
    for st in range(ST):
        th_dma_x(c, x, st, xns, "x")[0]()
    for kt in range(KT):
        for hh in range(3):
            f = wstage.tile([128, HIDDEN], F32, name="wqf", tag="wqf")
            nc.sync.dma_start(
                f[:], wq[kt * 128:(kt + 1) * 128,
                         hh * HIDDEN:(hh + 1) * HIDDEN])
            nc.vector.tensor_copy(
                c.wq16[kt][:, hh * HIDDEN:(hh + 1) * HIDDEN], f[:])
    bo_bcast = bass.AP(tensor=bo.tensor, offset=bo.offset,
                       ap=[[0, 128]] + list(bo.ap))
    nc.sync.dma_start(c.bias_sb[:], bo_bcast)

    # transposes of x (PE) as tiles arrive
    for st in range(ST):
        for f in th_xt(c, xT, st, xns, "x"):
            f()
    # prime pair-0 k and q (both halves) so the exp stream can start
    for f in th_qk(c, xT, 0, HIDDEN, kT_x) + th_qk(c, xT, 0, 0, qT_x):
        f()

    # stage Wout + x2 DMAs (issued now; consumed by aux thunks later)
    for ct in range(KT):
        f = wstage.tile([128, HIDDEN], F32, name="wof", tag="wqf")
        nc.sync.dma_start(f[:], wo[ct * 128:(ct + 1) * 128, :])
        nc.vector.tensor_copy(c.wout16[ct][:], f[:])
    for st in range(ST):
        th_dma_x(c, x2, st, xns, "x2")[0]()
    wstage.release()

    def den_tiles():
        return [c.dnp.tile([128, S], F16, name="den4", tag=f"den4{g}")
                for g in range(3)]

    den_1 = den_tiles()

    # ---------------- attn1 = attn(q, k, v) ---------------------------
    work1 = []
    for st in range(ST):
        work1 += th_v_half(c, xT, v_x, st, 0)
    for p in range(1, 3):
        work1 += th_qk(c, xT, p, HIDDEN, kT_x) + th_qk(c, xT, p, 0, qT_x)
    for st in range(ST):
        work1 += th_v_half(c, xT, v_x, st, 1)
    for p in range(3, NPAIR):
        work1 += th_qk(c, xT, p, HIDDEN, kT_x) + th_qk(c, xT, p, 0, qT_x)
    for st in range(ST):
        work1 += th_xt(c, x2T, st, xns, "x2")
    for p in range(NPAIR):
        work1 += th_qk(c, x2T, p, HIDDEN, kT_x2)
    for st in range(ST):
        work1 += th_v_half(c, x2T, v_x2, st, 0)
    attention(c, qT_x, kT_x, v_x, oT1, den_1, work1, [])

    # ---------------- attn3 = attn(q, k2, v2) -------------------------
    oT3 = [otp.tile([128, S], F16, name=f"oTb{i}", tag=f"oTb{i}")
           for i in range(NPAIR)]
    den_3 = den_tiles()
    work3 = []
    for st in range(ST):
        work3 += th_v_half(c, x2T, v_x2, st, 1)
    work3 += th_norm(c, oT1, den_1, 1)
    for p in range(NPAIR):
        work3 += th_qk(c, x2T, p, 0, qT_x2)
    for st in range(ST):
        work3 += th_proj(c, oT1, y1, st)
    attention(c, qT_x, kT_x2, v_x2, oT3, den_3, work3, [])
    xtp.release()
    wqp.release()
    kvxp.release()

    # ---------------- attn2 = attn(q2, k2, v2) ------------------------
    oT2 = [otp.tile([128, S], F16, name=f"oTa{i}", tag=f"oTa{i}")
           for i in range(NPAIR)]
    den_2 = den_tiles()
    work2 = []
    work2 += th_norm(c, oT3, den_3, 3)
    for st in range(ST):
        work2 += th_proj(c, oT3, y3, st)
    attention(c, qT_x2, kT_x2, v_x2, oT2, den_2, work2, [])
    x2tp.release()

    # ---------------- tail: normalize + proj y2 -----------------------
    for f in th_norm(c, oT2, den_2, 2):
        f()
    for st in range(ST):
        for f in th_proj(c, oT2, y2, st):
            f()

    c.ysb.release()
    c.bcsp.release()
    c.dnp.release()
    c.exps.release()
    c.xnat.release()
    c.auxp.release()
    c.ovps.release()
    c.sps.release()


def build_bass():
    from contextlib import ExitStack
    nc = bacc.Bacc("TRN2", target_bir_lowering=False, debug=False,
                   num_devices=B)
    x = nc.dram_tensor("x", [S, HIDDEN], F32, kind="ExternalInput").ap()
    x2 = nc.dram_tensor("x2", [S, HIDDEN], F32, kind="ExternalInput").ap()
    wq = nc.dram_tensor("Wqkv", [HIDDEN, 3 * HIDDEN], F32,
                        kind="ExternalInput").ap()
    wo = nc.dram_tensor("Wout", [HIDDEN, HIDDEN], F32,
                        kind="ExternalInput").ap()
    bo = nc.dram_tensor("bout", [HIDDEN], F32, kind="ExternalInput").ap()
    y1 = nc.dram_tensor("y1", [S, HIDDEN], F32, kind="ExternalOutput").ap()
    y2 = nc.dram_tensor("y2", [S, HIDDEN], F32, kind="ExternalOutput").ap()
    y3 = nc.dram_tensor("y3", [S, HIDDEN], F32, kind="ExternalOutput").ap()
    with tile.TileContext(nc) as tc:
        with ExitStack() as ctx:
            build_kernel(ctx, tc, x, x2, wq, wo, bo, y1, y2, y3)
    nc.compile()
    return nc


_NC_CACHE = []


def kernel(x, x2, Wqkv, Wout, bout):
    if not _NC_CACHE:
        _NC_CACHE.append(build_bass())
    nc = _NC_CACHE[0]
    in_maps = [
        {"x": np.ascontiguousarray(x[b]), "x2": np.ascontiguousarray(x2[b]),
         "Wqkv": Wqkv, "Wout": Wout, "bout": bout}
        for b in range(B)
    ]
    res = run_bass_kernel_spmd(nc, in_maps, list(range(B)))
    y1 = np.stack([res.results[b]["y1"] for b in range(B)])
    y2 = np.stack([res.results[b]["y2"] for b in range(B)])
    y3 = np.stack([res.results[b]["y3"] for b in range(B)])
    return (y1, y2, y3)


# revision 17
# speedup vs baseline: 1.2901x; 1.0198x over previous
"""Trainium2 Bass kernel for CDAttnBlock (v2 — pipelined).

Reference computation (per batch element b, all in fp32):
    q,k,v   = split(x  @ Wqkv)   heads=12, d=64
    q2,k2,v2= split(x2 @ Wqkv)
    o1 = attn(q, k,  v);  o2 = attn(q2, k2, v2);  o3 = attn(q, k2, v2)
    y_i = merge(o_i) @ Wout + bout

Sharding: pure data-parallel over batch (B=8) across 8 NeuronCores.

v2 design (vs v1 baseline at ~700us):
  - ScalarE exp stream (288 ACTs x ~1.15us = the metronome) starts ~30us
    in (right after x DMA + xT + pair-0 q/k priming) and never drains
    between attention phases: all other work (qkv of both inputs, x2
    transposes, output projections, softmax normalization) is pumped as
    small thunks into the per-step gaps.
  - Scores are computed per (pair, q-half, key-tile) with the two heads
    of a pair ROW-TILED onto disjoint PE quadrants (contraction d=64 at
    partitions 0-63 / 64-127) so both heads' score matmuls run
    concurrently; one [128,1024] PSUM tile holds [headA | headB] halves
    and one exp covers both.
  - av keeps the ones-column trick (lhsT [128, 65], row 64 = softmax
    denominator) accumulating over key-tiles into [65, 512] PSUM.
  - Normalize: denominators for all 12 heads collected into one
    [12, 1024] f16 tile -> ONE DVE reciprocal per attention (recip is
    ~6.4 cyc/elem so batching frees ~40us) -> gpsimd broadcast ->
    in-place f16 multiply on oT.
  - PSUM: scores 2x[128,1024] (4 banks) + ov 2x[65,512] (2) + aux 2x
    [128,512] (2) = 8 banks.
"""

import numpy as np

import concourse.bass as bass
import concourse.tile as tile
from concourse import bacc, mybir
from concourse.bass_utils import run_bass_kernel_spmd
from concourse.masks import make_identity

F32 = mybir.dt.float32
F16 = mybir.dt.float16
AF = mybir.ActivationFunctionType

HIDDEN = 768
HEADS = 12
D = 64
S = 1024
B = 8
SCALE = D ** -0.5
NPAIR = HEADS // 2          # 6 head pairs
KT = HIDDEN // 128          # 6 k-tiles over hidden
ST = S // 128               # 8 s-tiles
VW = D + 1                  # 65: v columns + ones column


class Ctx:
    """Shared handles for the kernel builder."""


# ---------------------------------------------------------------------------
# aux thunk builders (each returns a list of closures; every closure is a
# small burst of engine work suitable for pumping into exp-stream gaps)
# ---------------------------------------------------------------------------

def th_dma_x(c, x_ap, st, xns, key):
    nc = c.nc

    def f():
        xn = c.xnat.tile([128, HIDDEN], F32, name="xn", tag="xn")
        xns[(key, st)] = xn
        nc.sync.dma_start(xn[:], x_ap[st * 128:(st + 1) * 128, :])
    return [f]


def th_xt(c, xT, st, xns, key):
    """PE-transpose one s-tile of x into xT [128, KT*S]; two thunks."""
    nc = c.nc
    out3 = xT.rearrange("p (h s) -> p h s", s=S)

    def tp(half):
        def f():
            pt = c.auxp.tile([128, 3 * 128], F32, name="tpp",
                             tag=c.aux_tag())
            for i in range(3):
                ht = 3 * half + i
                nc.tensor.transpose(
                    pt[:, i * 128:(i + 1) * 128],
                    xns[(key, st)][:, ht * 128:(ht + 1) * 128], c.ident[:])
            nc.vector.tensor_copy(
                out3[:, 3 * half:3 * half + 3, st * 128:(st + 1) * 128],
                pt.rearrange("p (h s) -> p h s", s=128))
        return f
    return [tp(0), tp(1)]


def th_v_half(c, xT, v_st, st, half):
    """v columns for heads [0..8) (half 0) or [8..12) (half 1) of s-tile."""
    nc = c.nc

    def f():
        lo, hi = (0, 512) if half == 0 else (512, 768)
        vp = c.auxp.tile([128, hi - lo], F32, name="vp", tag=c.aux_tag())
        for kt in range(KT):
            nc.tensor.matmul(
                vp[:], xT[:, kt * S + st * 128:kt * S + (st + 1) * 128],
                c.wq16[kt][:, 2 * HIDDEN + lo:2 * HIDDEN + hi],
                start=(kt == 0), stop=(kt == KT - 1))
        vs3 = v_st[st].rearrange("p (h w) -> p h w", w=VW)
        ha, hb = (0, 8) if half == 0 else (8, 12)
        nc.vector.tensor_copy(
            vs3[:, ha:hb, 0:D], vp.rearrange("p (h w) -> p h w", w=D))
        if half == 1:
            nc.vector.tensor_copy(
                vs3[:, :, D:VW],
                c.onescol[:, None, :].broadcast_to([128, HEADS, 1]))
    return [f]


def th_qk(c, xT, p, base, dst):
    """qT or kT for one pair: two half-thunks, each [128, 512] psum."""
    nc = c.nc

    def half(hf):
        def f():
            pp = c.auxp.tile([128, 512], F32, name="qkp", tag=c.aux_tag())
            lo = hf * 512
            for kt in range(KT):
                nc.tensor.matmul(
                    pp[:],
                    c.wq16[kt][:, base + p * 128:base + (p + 1) * 128],
                    xT[:, kt * S + lo:kt * S + lo + 512],
                    start=(kt == 0), stop=(kt == KT - 1))
            nc.vector.tensor_copy(dst[p][:, lo:lo + 512], pp[:])
        return f
    return [half(0), half(1)]


def th_proj(c, oT, y_dram, st):
    """y[st] = oT.T @ Wout + bias -> DRAM; two half-thunks."""
    nc = c.nc
    hold = {}

    def half(h):
        def f():
            lo, hi = (0, 512) if h == 0 else (512, 768)
            yp = c.auxp.tile([128, hi - lo], F32, name="yp", tag=c.aux_tag())
            for ct in range(KT):
                nc.tensor.matmul(
                    yp[:], oT[ct][:, st * 128:(st + 1) * 128],
                    c.wout16[ct][:, lo:hi],
                    start=(ct == 0), stop=(ct == KT - 1))
            if h == 0:
                hold["yt"] = c.ysb.tile([128, HIDDEN], F32, name="yt",
                                        tag="yt")
            yt = hold["yt"]
            nc.vector.tensor_add(yt[:, lo:hi], yp[:], c.bias_sb[:, lo:hi])
            if h == 1:
                nc.sync.dma_start(y_dram[st * 128:(st + 1) * 128, :], yt[:])
        return f
    return [half(0), half(1)]


def th_norm(c, oT, den4s, attn_id):
    """Deferred normalize chain for one attention (12 heads in 3 groups
    of 4): per group, upcast + fast-approx reciprocal + downcast, then
    per-head gpsimd bcast + in-place f16 multiply on oT."""
    nc = c.nc
    hold = {}

    def t_recip(g):
        def f():
            df = c.dnp.tile([128, S], F32, name="df", tag="df", bufs=1)
            nc.vector.tensor_copy(df[:], den4s[g][:])
            nc.vector.reciprocal_approx_fast(df[:], df[:])
            r16 = c.dnp.tile([128, S], F16, name="r16", tag=f"r16{g}",
                             bufs=1)
            nc.vector.tensor_copy(r16[:], df[:])
            hold[g] = r16
        return f

    def t_head(h):
        def f():
            p, hh = h // 2, h % 2
            hp = slice(hh * D, (hh + 1) * D)
            g, j = h // 4, h % 4
            # partition_broadcast only supports src/dst partition 0, so
            # stage the recip row down to partition 0, then broadcast to
            # all 128 and multiply against the matching half (tensor ops
            # need equal input base partitions).
            rrow = c.bcsp.tile([1, S], F16, name="rrow", tag="rrow",
                               bufs=2)
            nc.vector.tensor_copy(rrow[:], hold[g][32 * j:32 * j + 1, :])
            bcs = c.bcsp.tile([128, S], F16, name="bcs", tag="bcs")
            nc.gpsimd.partition_broadcast(bcs[:], rrow[:])
            nc.vector.tensor_mul(oT[p][hp, :], oT[p][hp, :], bcs[hp, :])
        return f

    out = []
    for g in range(3):
        out.append(t_recip(g))
        out += [t_head(4 * g + j) for j in range(4)]
    return out


# ---------------------------------------------------------------------------
# the attention pipeline
# ---------------------------------------------------------------------------

def attention(c, qT, kT, v_st, oT, den4s, work, dveq):
    """One attention (12 heads as 6 row-tiled pairs x 2 q-halves x 8
    key-tiles). `work` = PE-ish aux thunks, `dveq` = deferred normalize
    chains of the previous attention; both are pumped into the gaps."""
    nc = c.nc
    state = {"i": 0, "credit": 0.0, "j": 0}
    sites = NPAIR * 2 * ST
    rate = max(0.001, len(work) / sites)

    def pump(k=1.0):
        state["credit"] += k * rate
        while state["credit"] >= 1.0 and state["i"] < len(work):
            work[state["i"]]()
            state["i"] += 1
            state["credit"] -= 1.0

    def pump_dve(n=1):
        for _ in range(n):
            if state["j"] < len(dveq):
                dveq[state["j"]]()
                state["j"] += 1

    for pair in range(NPAIR):
        for qh in range(2):
            qsl = slice(qh * 512, (qh + 1) * 512)
            ovA = c.ovps.tile([VW, 512], F32, name="ovA", tag="ovA")
            ovB = c.ovps.tile([VW, 512], F32, name="ovB", tag="ovB")
            for kt in range(ST):
                sp = c.sps.tile([128, S], F32, name="sp", tag="sp")
                ksl = slice(kt * 128, (kt + 1) * 128)
                nc.tensor.matmul(sp[:, 0:512], kT[pair][0:D, ksl],
                                 qT[pair][0:D, qsl], start=True, stop=True)
                nc.tensor.matmul(sp[:, 512:1024], kT[pair][D:128, ksl],
                                 qT[pair][D:128, qsl], start=True, stop=True)
                ex = c.exps.tile([128, S], F16, name="ex", tag="ex")
                nc.scalar.activation(ex[:], sp[:], AF.Exp,
                                     bias=c.zbias[:], scale=SCALE)
                vs3 = v_st[kt].rearrange("q (h w) -> q h w", w=VW)
                nc.tensor.matmul(ovA[:], vs3[:, 2 * pair, :], ex[:, 0:512],
                                 start=(kt == 0), stop=(kt == ST - 1))
                nc.tensor.matmul(ovB[:], vs3[:, 2 * pair + 1, :],
                                 ex[:, 512:1024],
                                 start=(kt == 0), stop=(kt == ST - 1))
                if kt < 4:
                    pump_dve(1)
                pump(1.0)
            # sweep tail: evacuate o (f16, pre-normalize) + denominators
            # (den row h goes to partition 32*(h%4) of group tile h//4 —
            # DVE moves must keep partition start congruent mod 32)
            hA, hB = 2 * pair, 2 * pair + 1
            nc.vector.tensor_copy(oT[pair][0:D, qsl], ovA[0:D, :])
            nc.vector.tensor_copy(
                den4s[hA // 4][32 * (hA % 4):32 * (hA % 4) + 1, qsl],
                ovA[D:VW, :])
            nc.vector.tensor_copy(oT[pair][D:128, qsl], ovB[0:D, :])
            nc.vector.tensor_copy(
                den4s[hB // 4][32 * (hB % 4):32 * (hB % 4) + 1, qsl],
                ovB[D:VW, :])
    # drain leftovers
    while state["i"] < len(work):
        work[state["i"]]()
        state["i"] += 1
    while state["j"] < len(dveq):
        dveq[state["j"]]()
        state["j"] += 1


def build_kernel(ctx, tc, x, x2, wq, wo, bo, y1, y2, y3):
    nc = tc.nc
    c = Ctx()
    c.nc = nc
    c._aux_flip = [0]

    def aux_tag():
        c._aux_flip[0] ^= 1
        return ("auxA", "auxB")[c._aux_flip[0]]
    c.aux_tag = aux_tag

    # ---------------- constants ---------------------------------------
    const = ctx.enter_context(tc.tile_pool(name="const", bufs=1))
    c.ident = const.tile([128, 128], F32, name="ident")
    make_identity(nc, c.ident)
    c.zbias = const.tile([128, 1], F32, name="zbias")
    nc.vector.memset(c.zbias[:], 0.0)
    c.onescol = const.tile([128, 1], F32, name="onescol")
    nc.vector.memset(c.onescol[:], 1.0)
    c.bias_sb = const.tile([128, HIDDEN], F32, name="bias_sb")

    # ---------------- persistent pools --------------------------------
    woutp = ctx.enter_context(tc.tile_pool(name="woutp", bufs=1))
    c.wout16 = [woutp.tile([128, HIDDEN], F16, name=f"wout{ct}",
                           tag=f"wout{ct}") for ct in range(KT)]
    qxp = ctx.enter_context(tc.tile_pool(name="qxp", bufs=1))
    qT_x = [qxp.tile([128, S], F16, name=f"qTx{i}", tag=f"qTx{i}")
            for i in range(NPAIR)]
    kvx2p = ctx.enter_context(tc.tile_pool(name="kvx2p", bufs=1))
    qT_x2 = [kvx2p.tile([128, S], F16, name=f"qTx2{i}", tag=f"qTx2{i}")
             for i in range(NPAIR)]
    kT_x2 = [kvx2p.tile([128, S], F16, name=f"kTx2{i}", tag=f"kTx2{i}")
             for i in range(NPAIR)]
    v_x2 = [kvx2p.tile([128, HEADS * VW], F16, name=f"vx2{i}",
                       tag=f"vx2{i}") for i in range(ST)]
    otp = ctx.enter_context(tc.tile_pool(name="otp", bufs=1))
    oT1 = [otp.tile([128, S], F16, name=f"oTa{i}", tag=f"oTa{i}")
           for i in range(NPAIR)]

    # pools released mid-build (allocated after the persistent ones)
    x2tp = tc.alloc_tile_pool(name="x2tp", bufs=1)
    x2T = x2tp.tile([128, KT * S], F16, name="x2T")
    kvxp = tc.alloc_tile_pool(name="kvxp", bufs=1)
    kT_x = [kvxp.tile([128, S], F16, name=f"kTx{i}", tag=f"kTx{i}")
            for i in range(NPAIR)]
    v_x = [kvxp.tile([128, HEADS * VW], F16, name=f"vx{i}", tag=f"vx{i}")
           for i in range(ST)]
    wqp = tc.alloc_tile_pool(name="wqp", bufs=1)
    c.wq16 = [wqp.tile([128, 3 * HIDDEN], F16, name=f"wq16{kt}",
                       tag=f"wq16{kt}") for kt in range(KT)]
    xtp = tc.alloc_tile_pool(name="xtp", bufs=1)
    xT = xtp.tile([128, KT * S], F16, name="xT")

    # ---------------- working pools (right side) ----------------------
    c.xnat = tc.alloc_tile_pool(name="xnat", bufs=2, side="right")
    c.exps = tc.alloc_tile_pool(name="exps", bufs=2, side="right")
    c.dnp = tc.alloc_tile_pool(name="dnp", bufs=2, side="right")
    c.bcsp = tc.alloc_tile_pool(name="bcsp", bufs=1, side="right")
    wstage = tc.alloc_tile_pool(name="wstage", bufs=2, side="right")

    # ---------------- PSUM pools --------------------------------------
    c.sps = tc.alloc_tile_pool(name="sps", bufs=2, space="PSUM")
    c.ovps = tc.alloc_tile_pool(name="ovps", bufs=1, space="PSUM")
    c.auxp = tc.alloc_tile_pool(name="auxp", bufs=1, space="PSUM")

    # ---------------- lead-in: DMA order + priming --------------------
    # x first (gates everything), then Wqkv, then Wout/bias, then x2.
    xns = {}
    for st in range(ST):
        th_dma_x(c, x, st, xns, "x")[0]()
    for kt in range(KT):
        for hh in range(3):
            f = wstage.tile([128, HIDDEN], F32, name="wqf", tag="wqf")
            nc.sync.dma_start(
                f[:], wq[kt * 128:(kt + 1) * 128,
                         hh * HIDDEN:(hh + 1) * HIDDEN])
            nc.vector.tensor_copy(
                c.wq16[kt][:, hh * HIDDEN:(hh + 1) * HIDDEN], f[:])
    bo_bcast = bass.AP(tensor=bo.tensor, offset=bo.offset,
                       ap=[[0, 128]] + list(bo.ap))
    nc.sync.dma_start(c.bias_sb[:], bo_bcast)

    # transposes of x (PE) as tiles arrive
    for st in range(ST):
        for f in th_xt(c, xT, st, xns, "x"):
            f()
    # prime pair-0 k and q (both halves) so the exp stream can start
    for f in th_qk(c, xT, 0, HIDDEN, kT_x) + th_qk(c, xT, 0, 0, qT_x):
        f()

    # stage Wout + x2 DMAs (issued now; consumed by aux thunks later)
    for ct in range(KT):
        f = wstage.tile([128, HIDDEN], F32, name="wof", tag="wqf")
        nc.sync.dma_start(f[:], wo[ct * 128:(ct + 1) * 128, :])
        nc.vector.tensor_copy(c.wout16[ct][:], f[:])
    for st in range(ST):
        th_dma_x(c, x2, st, xns, "x2")[0]()
    wstage.release()
    c.ysb = tc.alloc_tile_pool(name="ysb", bufs=2, side="right")

    def den_tiles():
        return [c.dnp.tile([128, S], F16, name="den4", tag=f"den4{g}")
                for g in range(3)]

    den_1 = den_tiles()

    # ---------------- attn1 = attn(q, k, v) ---------------------------
    work1 = []
    for st in range(ST):
        work1 += th_v_half(c, xT, v_x, st, 0)
    for p in range(1, 3):
        work1 += th_qk(c, xT, p, HIDDEN, kT_x) + th_qk(c, xT, p, 0, qT_x)
    for st in range(ST):
        work1 += th_v_half(c, xT, v_x, st, 1)
    for p in range(3, NPAIR):
        work1 += th_qk(c, xT, p, HIDDEN, kT_x) + th_qk(c, xT, p, 0, qT_x)
    for st in range(ST):
        work1 += th_xt(c, x2T, st, xns, "x2")
    for p in range(NPAIR):
        work1 += th_qk(c, x2T, p, HIDDEN, kT_x2)
    for st in range(ST):
        work1 += th_v_half(c, x2T, v_x2, st, 0)
    attention(c, qT_x, kT_x, v_x, oT1, den_1, work1, [])

    # ---------------- attn3 = attn(q, k2, v2) -------------------------
    oT3 = [otp.tile([128, S], F16, name=f"oTb{i}", tag=f"oTb{i}")
           for i in range(NPAIR)]
    den_3 = den_tiles()
    work3 = []
    for st in range(ST):
        work3 += th_v_half(c, x2T, v_x2, st, 1)
    work3 += th_norm(c, oT1, den_1, 1)
    for p in range(NPAIR):
        work3 += th_qk(c, x2T, p, 0, qT_x2)
    for st in range(ST):
        work3 += th_proj(c, oT1, y1, st)
    attention(c, qT_x, kT_x2, v_x2, oT3, den_3, work3, [])
    xtp.release()
    wqp.release()
    kvxp.release()

    # ---------------- attn2 = attn(q2, k2, v2) ------------------------
    oT2 = [otp.tile([128, S], F16, name=f"oTa{i}", tag=f"oTa{i}")
           for i in range(NPAIR)]
    den_2 = den_tiles()
    work2 = []
    work2 += th_norm(c, oT3, den_3, 3)
    for st in range(ST):
        work2 += th_proj(c, oT3, y3, st)
    attention(c, qT_x2, kT_x2, v_x2, oT2, den_2, work2, [])
    x2tp.release()

    # ---------------- tail: normalize + proj y2 -----------------------
    for f in th_norm(c, oT2, den_2, 2):
        f()
    for st in range(ST):
        for f in th_proj(c, oT2, y2, st):
            f()

    c.ysb.release()
    c.bcsp.release()
    c.dnp.release()
    c.exps.release()
    c.xnat.release()
    c.auxp.release()
    c.ovps.release()
    c.sps.release()


def build_bass():
    from contextlib import ExitStack
    nc = bacc.Bacc("TRN2", target_bir_lowering=False, debug=False,
                   num_devices=B)
    x = nc.dram_tensor("x", [S, HIDDEN], F32, kind="ExternalInput").ap()
    x2 = nc.dram_tensor("x2", [S, HIDDEN], F32, kind="ExternalInput").ap()
    wq = nc.dram_tensor("Wqkv", [HIDDEN, 3 * HIDDEN], F32,
                        kind="ExternalInput").ap()
    wo = nc.dram_tensor("Wout", [HIDDEN, HIDDEN], F32,
                        kind="ExternalInput").ap()
    bo = nc.dram_tensor("bout", [HIDDEN], F32, kind="ExternalInput").ap()
    y1 = nc.dram_tensor("y1", [S, HIDDEN], F32, kind="ExternalOutput").ap()
    y2 = nc.dram_tensor("y2", [S, HIDDEN], F32, kind="ExternalOutput").ap()
    y3 = nc.dram_tensor("y3", [S, HIDDEN], F32, kind="ExternalOutput").ap()
    with tile.TileContext(nc) as tc:
        with ExitStack() as ctx:
            build_kernel(ctx, tc, x, x2, wq, wo, bo, y1, y2, y3)
    nc.compile()
    return nc


_NC_CACHE = []


def kernel(x, x2, Wqkv, Wout, bout):
    if not _NC_CACHE:
        _NC_CACHE.append(build_bass())
    nc = _NC_CACHE[0]
    in_maps = [
        {"x": np.ascontiguousarray(x[b]), "x2": np.ascontiguousarray(x2[b]),
         "Wqkv": Wqkv, "Wout": Wout, "bout": bout}
        for b in range(B)
    ]
    res = run_bass_kernel_spmd(nc, in_maps, list(range(B)))
    y1 = np.stack([res.results[b]["y1"] for b in range(B)])
    y2 = np.stack([res.results[b]["y2"] for b in range(B)])
    y3 = np.stack([res.results[b]["y3"] for b in range(B)])
    return (y1, y2, y3)
